# revision 3
# baseline (speedup 1.0000x reference)
"""ClusterScaleBiasBlock Trainium2 kernel.

Computes out = BN(x) * (1 + Wg[ids]) + Wb[ids] for
x:[32768,2048] f32, Wg/Wb:[64,2048], ids:[32768] int32, where
BN(x) = (x - mean) * rsqrt(var+eps) * gamma + beta (inference mode).

Algebraic folding (host side, tiny [64,2048] tables):
    inv  = rsqrt(var + eps) * gamma
    S[c] = inv * (1 + Wg[c])
    T[c] = (beta - mean*inv) * (1 + Wg[c]) + Wb[c]
    out  = x * S[ids] + T[ids]

Layout strategy (the kernel is HBM-bandwidth-bound, so minimize traffic):
  - Shard by CLUSTER, not by batch row: core c owns 8 whole clusters, so
    every row a core touches uses one of 8 (scale, bias) vector pairs.
  - Clusters are rank-matched into 8 "slots" (slot j = clusters with
    size-rank 8j..8j+7, one per core) and each slot is padded to the max
    size in its octile -> all 8 cores share identical slot extents, so a
    single SPMD program works; padding is only ~1-2%.
  - Host transposes x rows into feature-major [2048, R] bf16 tiles.  With
    features on partitions, out = x*s + t needs just ONE VectorE
    tensor_scalar instruction per (feature-tile, slot): s,t are
    per-partition scalar APs.  bf16 + unit stride -> DVE 4x mode.
  - x/out move as bf16 (tolerance is 2e-2; bf16 gives ~4e-3), halving
    HBM traffic vs f32: ~34 MB/core total vs 64 MB for the f32 baseline.
  - Loads ride the SP HWDGE ring, stores the ACT ring.
"""

import sys

if "/opt/trn_rl_repo" not in sys.path:
    sys.path.insert(0, "/opt/trn_rl_repo")

import numpy as np

B, F, C = 32768, 2048, 64
N_CORES = 8
P = 128
NFT = F // P          # 16 feature tiles of 128 partitions
BN_EPS = 1e-3

_PROGRAM = None
_PROG_KEY = None


def _build_program(R, M):
    """R = padded rows per core; M = per-slot column extents (sum == R)."""
    import concourse.bacc as bacc
    import concourse.mybir as mybir
    from concourse import tile

    f32 = mybir.dt.float32
    bf16 = mybir.dt.bfloat16
    nslots = len(M)
    q = [0]
    for m in M:
        q.append(q[-1] + m)

    nc = bacc.Bacc(None)
    x_d = nc.declare_dram_parameter("xt", [F, R], bf16, isOutput=False)
    s_d = nc.declare_dram_parameter("stab", [P, NFT * nslots], f32, isOutput=False)
    t_d = nc.declare_dram_parameter("ttab", [P, NFT * nslots], f32, isOutput=False)
    o_d = nc.declare_dram_parameter("ot", [F, R], bf16, isOutput=True)

    mult = mybir.AluOpType.mult
    add = mybir.AluOpType.add

    # Alternate loads and stores across the two HWDGE rings (SP=sync,
    # ACT=scalar) so both rings carry traffic from the first tile to the
    # last: otherwise the load ring idles during the store-only drain and
    # vice versa, which costs ~10us on the HBM-stack-shared cores.
    h = q[(nslots + 1) // 2]  # split point (even column, ~half the tile)
    with tile.TileContext(nc) as tc:
        with (
            tc.tile_pool(name="const", bufs=1) as cpool,
            tc.tile_pool(name="xin", bufs=3) as xpool,
            tc.tile_pool(name="oout", bufs=3) as opool,
        ):
            s_sb = cpool.tile([P, NFT * nslots], f32, tag="stab")
            t_sb = cpool.tile([P, NFT * nslots], f32, tag="ttab")
            nc.scalar.dma_start(out=s_sb[:], in_=s_d[:])
            nc.scalar.dma_start(out=t_sb[:], in_=t_d[:])

            for ft in range(NFT):
                ld = nc.sync if ft % 2 == 0 else nc.scalar
                st = nc.scalar if ft % 2 == 0 else nc.sync
                rows = slice(ft * P, (ft + 1) * P)
                xt = xpool.tile([P, R], bf16, tag="x")
                # half-granularity transfers: compute on the first half
                # starts while the second half is in flight, and the two
                # rings stay near-saturated through the head and tail.
                ld.dma_start(out=xt[:, 0:h], in_=x_d[rows, 0:h])
                ld.dma_start(out=xt[:, h:], in_=x_d[rows, h:])
                ot = opool.tile([P, R], bf16, tag="o")
                for j in range(nslots):
                    cs = slice(q[j], q[j + 1])
                    col = ft * nslots + j
                    nc.vector.tensor_scalar(
                        ot[:, cs], xt[:, cs],
                        s_sb[:, col:col + 1], t_sb[:, col:col + 1],
                        mult, add)
                st.dma_start(out=o_d[rows, 0:h], in_=ot[:, 0:h])
                st.dma_start(out=o_d[rows, h:], in_=ot[:, h:])
    nc.compile()
    return nc


def _host_tables(Wg, Wb, bn_gamma, bn_beta, moving_mean, moving_var):
    inv = (bn_gamma.astype(np.float64)
           / np.sqrt(moving_var.astype(np.float64) + BN_EPS))
    gp1 = 1.0 + Wg.astype(np.float64)  # [C, F]
    S = (inv[None, :] * gp1).astype(np.float32)
    T = ((bn_beta.astype(np.float64) - moving_mean.astype(np.float64) * inv)[None, :]
         * gp1 + Wb.astype(np.float64)).astype(np.float32)
    return S, T


def kernel(x, Wg, Wb, bn_gamma, bn_beta, moving_mean, moving_var, cluster_ids):
    global _PROGRAM, _PROG_KEY
    import ml_dtypes
    from concourse.bass_utils import run_bass_kernel_spmd

    bf16 = ml_dtypes.bfloat16
    x = np.asarray(x, dtype=np.float32)
    ids = np.asarray(cluster_ids, dtype=np.int32)
    S, T = _host_tables(
        np.asarray(Wg, np.float32), np.asarray(Wb, np.float32),
        np.asarray(bn_gamma, np.float32), np.asarray(bn_beta, np.float32),
        np.asarray(moving_mean, np.float32), np.asarray(moving_var, np.float32),
    )

    counts = np.bincount(ids, minlength=C)
    present = np.nonzero(counts)[0]
    ranked = present[np.argsort(-counts[present], kind="stable")]
    npad = (-len(ranked)) % N_CORES
    ranked = np.concatenate([ranked, np.full(npad, -1, dtype=np.int64)])
    nslots = len(ranked) // N_CORES
    slot_cl = ranked.reshape(nslots, N_CORES)   # [slot, core] -> cluster id
    # slot extents: max cluster size in the octile, rounded up to even
    M = []
    for j in range(nslots):
        mx = max((int(counts[cl]) for cl in slot_cl[j] if cl >= 0), default=0)
        M.append(max(2, ((mx + 1) // 2) * 2))
    R = int(sum(M))
    q = [0]
    for m in M:
        q.append(q[-1] + m)

    order = np.argsort(ids, kind="stable")
    starts = np.zeros(C + 1, dtype=np.int64)
    np.cumsum(counts, out=starts[1:])

    in_maps = []
    idx_all, cnt_all = [], []
    for c in range(N_CORES):
        idx = np.empty(R, dtype=np.int64)
        cnts = np.zeros(nslots, dtype=np.int64)
        stab = np.zeros((F, nslots), dtype=np.float32)
        ttab = np.zeros((F, nslots), dtype=np.float32)
        for j in range(nslots):
            a, b = q[j], q[j + 1]
            cl = int(slot_cl[j, c])
            if cl < 0:
                idx[a:b] = order[0]     # scale/bias stay 0; never scattered
                continue
            n = int(counts[cl])
            rows = order[starts[cl]:starts[cl] + n]
            idx[a:a + n] = rows
            idx[a + n:b] = rows[0]      # pad with a row of the same cluster
            cnts[j] = n
            stab[:, j] = S[cl]
            ttab[:, j] = T[cl]
        idx_all.append(idx)
        cnt_all.append(cnts)
        # feature-major bf16 transpose of this core's rows: [F, R]
        xt = x[idx].T.astype(bf16, order="C")
        st = stab.reshape(NFT, P, nslots).transpose(1, 0, 2).reshape(P, NFT * nslots)
        tt = ttab.reshape(NFT, P, nslots).transpose(1, 0, 2).reshape(P, NFT * nslots)
        in_maps.append({
            "xt": xt,
            "stab": np.ascontiguousarray(st),
            "ttab": np.ascontiguousarray(tt),
        })

    key = (R, tuple(M))
    if _PROGRAM is None or _PROG_KEY != key:
        _PROGRAM = _build_program(R, M)
        _PROG_KEY = key

    res = run_bass_kernel_spmd(_PROGRAM, in_maps, list(range(N_CORES)))
    globals()["LAST_RESULT"] = res

    out = np.empty((B, F), dtype=np.float32)
    for c in range(N_CORES):
        z = np.asarray(res.results[c]["ot"])          # [F, R] bf16
        zf = z.T.astype(np.float32, order="C")        # [R, F]
        idx, cnts = idx_all[c], cnt_all[c]
        for j in range(nslots):
            a, n = q[j], int(cnts[j])
            if n:
                out[idx[a:a + n]] = zf[a:a + n]
    return out


if __name__ == "__main__":
    # Smoke test with random data against a local numpy reference.
    rng = np.random.default_rng(0)
    inputs = {
        "x": rng.standard_normal((B, F), dtype=np.float32),
        "Wg": 0.25 * rng.standard_normal((C, F)).astype(np.float32),
        "Wb": 0.25 * rng.standard_normal((C, F)).astype(np.float32),
        "bn_gamma": np.ones(F, np.float32),
        "bn_beta": np.zeros(F, np.float32),
        "moving_mean": 0.1 * rng.standard_normal(F).astype(np.float32),
        "moving_var": rng.uniform(0.5, 1.5, F).astype(np.float32),
        "cluster_ids": rng.integers(0, C, B, dtype=np.int32),
    }
    out = kernel(**inputs)
    inv = inputs["bn_gamma"] / np.sqrt(inputs["moving_var"] + BN_EPS)
    xn = (inputs["x"] - inputs["moving_mean"]) * inv + inputs["bn_beta"]
    g = inputs["Wg"][inputs["cluster_ids"]]
    b = inputs["Wb"][inputs["cluster_ids"]]
    ref = xn * (1.0 + g) + b
    err = np.max(np.abs(out - ref)) / np.max(np.abs(ref))
    print("rel err:", err)


# revision 4
# speedup vs baseline: 1.1212x; 1.1212x over previous
"""ClusterScaleBiasBlock Trainium2 kernel.

Computes out = BN(x) * (1 + Wg[ids]) + Wb[ids] for
x:[32768,2048] f32, Wg/Wb:[64,2048], ids:[32768] int32, where
BN(x) = (x - mean) * rsqrt(var+eps) * gamma + beta (inference mode).

Algebraic folding (host side, tiny [64,2048] tables):
    inv  = rsqrt(var + eps) * gamma
    S[c] = inv * (1 + Wg[c])
    T[c] = (beta - mean*inv) * (1 + Wg[c]) + Wb[c]
    out  = x * S[ids] + T[ids]

Layout strategy (the kernel is HBM-bandwidth-bound, so minimize traffic):
  - Shard by CLUSTER, not by batch row: core c owns 8 whole clusters, so
    every row a core touches uses one of 8 (scale, bias) vector pairs.
  - Clusters are rank-matched into 8 "slots" (slot j = clusters with
    size-rank 8j..8j+7, one per core) and each slot is padded to the max
    size in its octile -> all 8 cores share identical slot extents, so a
    single SPMD program works; padding is only ~1-2%.
  - Host transposes x rows into feature-major [2048, R] bf16 tiles.  With
    features on partitions, out = x*s + t needs just ONE VectorE
    tensor_scalar instruction per (feature-tile, slot): s,t are
    per-partition scalar APs.  bf16 + unit stride -> DVE 4x mode.
  - x/out move as bf16 (tolerance is 2e-2; bf16 gives ~4e-3), halving
    HBM traffic vs f32: ~34 MB/core total vs 64 MB for the f32 baseline.
  - Loads ride the SP HWDGE ring, stores the ACT ring.
"""

import sys

if "/opt/trn_rl_repo" not in sys.path:
    sys.path.insert(0, "/opt/trn_rl_repo")

import numpy as np

B, F, C = 32768, 2048, 64
N_CORES = 8
P = 128
NFT = F // P          # 16 feature tiles of 128 partitions
BN_EPS = 1e-3

_PROGRAM = None
_PROG_KEY = None


def _build_program(R, M):
    """R = padded rows per core; M = per-slot column extents (sum == R)."""
    import concourse.bacc as bacc
    import concourse.mybir as mybir
    from concourse import tile

    f32 = mybir.dt.float32
    bf16 = mybir.dt.bfloat16
    nslots = len(M)
    q = [0]
    for m in M:
        q.append(q[-1] + m)

    nc = bacc.Bacc(None)
    x_d = nc.declare_dram_parameter("xt", [F, R], bf16, isOutput=False)
    s_d = nc.declare_dram_parameter("stab", [P, NFT * nslots], f32, isOutput=False)
    t_d = nc.declare_dram_parameter("ttab", [P, NFT * nslots], f32, isOutput=False)
    o_d = nc.declare_dram_parameter("ot", [F, R], bf16, isOutput=True)

    mult = mybir.AluOpType.mult
    add = mybir.AluOpType.add

    # Each HWDGE ring (SP=sync, ACT=scalar) carries half the loads AND
    # half the stores, alternating by feature tile, so both rings stream
    # from the first tile to the last (a dedicated store ring would idle
    # for the first ~12us and a dedicated load ring for the last ~10us).
    # Transfers stay full-tile (~1 MB, 8 KB per partition line) -- halving
    # them measurably drops per-ring throughput.  Stores are emitted two
    # tiles behind loads so neither ring head-of-line blocks on compute.
    h = q[(nslots + 1) // 2]  # split point (even column, ~half the tile)
    with tile.TileContext(nc) as tc:
        with (
            tc.tile_pool(name="const", bufs=1) as cpool,
            tc.tile_pool(name="xin", bufs=4) as xpool,
            tc.tile_pool(name="oout", bufs=4) as opool,
        ):
            s_sb = cpool.tile([P, NFT * nslots], f32, tag="stab")
            t_sb = cpool.tile([P, NFT * nslots], f32, tag="ttab")
            nc.sync.dma_start(out=s_sb[:], in_=s_d[:])
            nc.scalar.dma_start(out=t_sb[:], in_=t_d[:])

            xts, ots = {}, {}

            def emit_load(ft):
                ld = nc.sync if ft % 2 == 0 else nc.scalar
                rows = slice(ft * P, (ft + 1) * P)
                xt = xpool.tile([P, R], bf16, tag="x")
                if ft < 2:
                    # split the first load on each ring: compute starts
                    # after half a tile is in
                    ld.dma_start(out=xt[:, 0:h], in_=x_d[rows, 0:h])
                    ld.dma_start(out=xt[:, h:], in_=x_d[rows, h:])
                else:
                    ld.dma_start(out=xt[:], in_=x_d[rows, :])
                xts[ft] = xt

            def emit_compute_store(ft):
                st = nc.scalar if ft % 2 == 0 else nc.sync
                rows = slice(ft * P, (ft + 1) * P)
                xt = xts.pop(ft)
                ot = opool.tile([P, R], bf16, tag="o")
                for j in range(nslots):
                    cs = slice(q[j], q[j + 1])
                    col = ft * nslots + j
                    nc.vector.tensor_scalar(
                        ot[:, cs], xt[:, cs],
                        s_sb[:, col:col + 1], t_sb[:, col:col + 1],
                        mult, add)
                if ft >= NFT - 2:
                    # split the last store on each ring: the drain is
                    # latency-bound once loads have finished
                    st.dma_start(out=o_d[rows, 0:h], in_=ot[:, 0:h])
                    st.dma_start(out=o_d[rows, h:], in_=ot[:, h:])
                else:
                    st.dma_start(out=o_d[rows, :], in_=ot[:])

            for ft in range(NFT + 2):
                if ft < NFT:
                    emit_load(ft)
                if ft >= 2:
                    emit_compute_store(ft - 2)
    nc.compile()
    return nc


def _host_tables(Wg, Wb, bn_gamma, bn_beta, moving_mean, moving_var):
    inv = (bn_gamma.astype(np.float64)
           / np.sqrt(moving_var.astype(np.float64) + BN_EPS))
    gp1 = 1.0 + Wg.astype(np.float64)  # [C, F]
    S = (inv[None, :] * gp1).astype(np.float32)
    T = ((bn_beta.astype(np.float64) - moving_mean.astype(np.float64) * inv)[None, :]
         * gp1 + Wb.astype(np.float64)).astype(np.float32)
    return S, T


def kernel(x, Wg, Wb, bn_gamma, bn_beta, moving_mean, moving_var, cluster_ids):
    global _PROGRAM, _PROG_KEY
    import ml_dtypes
    from concourse.bass_utils import run_bass_kernel_spmd

    bf16 = ml_dtypes.bfloat16
    x = np.asarray(x, dtype=np.float32)
    ids = np.asarray(cluster_ids, dtype=np.int32)
    S, T = _host_tables(
        np.asarray(Wg, np.float32), np.asarray(Wb, np.float32),
        np.asarray(bn_gamma, np.float32), np.asarray(bn_beta, np.float32),
        np.asarray(moving_mean, np.float32), np.asarray(moving_var, np.float32),
    )

    counts = np.bincount(ids, minlength=C)
    present = np.nonzero(counts)[0]
    ranked = present[np.argsort(-counts[present], kind="stable")]
    npad = (-len(ranked)) % N_CORES
    ranked = np.concatenate([ranked, np.full(npad, -1, dtype=np.int64)])
    nslots = len(ranked) // N_CORES
    slot_cl = ranked.reshape(nslots, N_CORES)   # [slot, core] -> cluster id
    # slot extents: max cluster size in the octile, rounded up to even
    M = []
    for j in range(nslots):
        mx = max((int(counts[cl]) for cl in slot_cl[j] if cl >= 0), default=0)
        M.append(max(2, ((mx + 1) // 2) * 2))
    R = int(sum(M))
    q = [0]
    for m in M:
        q.append(q[-1] + m)

    order = np.argsort(ids, kind="stable")
    starts = np.zeros(C + 1, dtype=np.int64)
    np.cumsum(counts, out=starts[1:])

    in_maps = []
    idx_all, cnt_all = [], []
    for c in range(N_CORES):
        idx = np.empty(R, dtype=np.int64)
        cnts = np.zeros(nslots, dtype=np.int64)
        stab = np.zeros((F, nslots), dtype=np.float32)
        ttab = np.zeros((F, nslots), dtype=np.float32)
        for j in range(nslots):
            a, b = q[j], q[j + 1]
            cl = int(slot_cl[j, c])
            if cl < 0:
                idx[a:b] = order[0]     # scale/bias stay 0; never scattered
                continue
            n = int(counts[cl])
            rows = order[starts[cl]:starts[cl] + n]
            idx[a:a + n] = rows
            idx[a + n:b] = rows[0]      # pad with a row of the same cluster
            cnts[j] = n
            stab[:, j] = S[cl]
            ttab[:, j] = T[cl]
        idx_all.append(idx)
        cnt_all.append(cnts)
        # feature-major bf16 transpose of this core's rows: [F, R]
        xt = x[idx].T.astype(bf16, order="C")
        st = stab.reshape(NFT, P, nslots).transpose(1, 0, 2).reshape(P, NFT * nslots)
        tt = ttab.reshape(NFT, P, nslots).transpose(1, 0, 2).reshape(P, NFT * nslots)
        in_maps.append({
            "xt": xt,
            "stab": np.ascontiguousarray(st),
            "ttab": np.ascontiguousarray(tt),
        })

    key = (R, tuple(M))
    if _PROGRAM is None or _PROG_KEY != key:
        _PROGRAM = _build_program(R, M)
        _PROG_KEY = key

    res = run_bass_kernel_spmd(_PROGRAM, in_maps, list(range(N_CORES)))
    globals()["LAST_RESULT"] = res

    out = np.empty((B, F), dtype=np.float32)
    for c in range(N_CORES):
        z = np.asarray(res.results[c]["ot"])          # [F, R] bf16
        zf = z.T.astype(np.float32, order="C")        # [R, F]
        idx, cnts = idx_all[c], cnt_all[c]
        for j in range(nslots):
            a, n = q[j], int(cnts[j])
            if n:
                out[idx[a:a + n]] = zf[a:a + n]
    return out


if __name__ == "__main__":
    # Smoke test with random data against a local numpy reference.
    rng = np.random.default_rng(0)
    inputs = {
        "x": rng.standard_normal((B, F), dtype=np.float32),
        "Wg": 0.25 * rng.standard_normal((C, F)).astype(np.float32),
        "Wb": 0.25 * rng.standard_normal((C, F)).astype(np.float32),
        "bn_gamma": np.ones(F, np.float32),
        "bn_beta": np.zeros(F, np.float32),
        "moving_mean": 0.1 * rng.standard_normal(F).astype(np.float32),
        "moving_var": rng.uniform(0.5, 1.5, F).astype(np.float32),
        "cluster_ids": rng.integers(0, C, B, dtype=np.int32),
    }
    out = kernel(**inputs)
    inv = inputs["bn_gamma"] / np.sqrt(inputs["moving_var"] + BN_EPS)
    xn = (inputs["x"] - inputs["moving_mean"]) * inv + inputs["bn_beta"]
    g = inputs["Wg"][inputs["cluster_ids"]]
    b = inputs["Wb"][inputs["cluster_ids"]]
    ref = xn * (1.0 + g) + b
    err = np.max(np.abs(out - ref)) / np.max(np.abs(ref))
    print("rel err:", err)


# revision 6
# speedup vs baseline: 1.1486x; 1.0244x over previous
"""ClusterScaleBiasBlock Trainium2 kernel.

Computes out = BN(x) * (1 + Wg[ids]) + Wb[ids] for
x:[32768,2048] f32, Wg/Wb:[64,2048], ids:[32768] int32, where
BN(x) = (x - mean) * rsqrt(var+eps) * gamma + beta (inference mode).

Algebraic folding (host side, tiny [64,2048] tables):
    inv  = rsqrt(var + eps) * gamma
    S[c] = inv * (1 + Wg[c])
    T[c] = (beta - mean*inv) * (1 + Wg[c]) + Wb[c]
    out  = x * S[ids] + T[ids]

Layout strategy (the kernel is HBM-bandwidth-bound, so minimize traffic):
  - Shard by CLUSTER, not by batch row: core c owns 8 whole clusters, so
    every row a core touches uses one of 8 (scale, bias) vector pairs.
  - Clusters are rank-matched into 8 "slots" (slot j = clusters with
    size-rank 8j..8j+7, one per core) and each slot is padded to the max
    size in its octile -> all 8 cores share identical slot extents, so a
    single SPMD program works; padding is only ~1-2%.
  - Host transposes x rows into feature-major [2048, R] bf16 tiles.  With
    features on partitions, out = x*s + t needs just ONE VectorE
    tensor_scalar instruction per (feature-tile, slot): s,t are
    per-partition scalar APs.  bf16 + unit stride -> DVE 4x mode.
  - x/out move as bf16 (tolerance is 2e-2; bf16 gives ~4e-3), halving
    HBM traffic vs f32: ~34 MB/core total vs 64 MB for the f32 baseline.
  - Loads ride the SP HWDGE ring, stores the ACT ring.
"""

import sys

if "/opt/trn_rl_repo" not in sys.path:
    sys.path.insert(0, "/opt/trn_rl_repo")

import numpy as np

B, F, C = 32768, 2048, 64
N_CORES = 8
P = 128
NFT = F // P          # 16 feature tiles of 128 partitions
BN_EPS = 1e-3

_PROGRAM = None
_PROG_KEY = None


def _build_program(R, M):
    """R = padded rows per core; M = per-slot column extents (sum == R)."""
    import concourse.bacc as bacc
    import concourse.mybir as mybir
    from concourse import tile

    f32 = mybir.dt.float32
    bf16 = mybir.dt.bfloat16
    nslots = len(M)
    q = [0]
    for m in M:
        q.append(q[-1] + m)

    nc = bacc.Bacc(None)
    x_d = nc.declare_dram_parameter("xt", [F, R], bf16, isOutput=False)
    s_d = nc.declare_dram_parameter("stab", [P, NFT * nslots], f32, isOutput=False)
    t_d = nc.declare_dram_parameter("ttab", [P, NFT * nslots], f32, isOutput=False)
    o_d = nc.declare_dram_parameter("ot", [F, R], bf16, isOutput=True)

    mult = mybir.AluOpType.mult
    add = mybir.AluOpType.add

    # Each HWDGE ring (SP=sync, ACT=scalar) carries half the loads AND
    # half the stores, alternating by feature tile, so both rings stream
    # from the first tile to the last (a dedicated store ring would idle
    # for the first ~12us and a dedicated load ring for the last ~10us).
    # Transfers stay full-tile (~1 MB, 8 KB per partition line) -- halving
    # them measurably drops per-ring throughput.  Stores are emitted two
    # tiles behind loads so neither ring head-of-line blocks on compute.
    h = q[(nslots + 1) // 2]  # split point (even column, ~half the tile)
    with tile.TileContext(nc) as tc:
        with (
            tc.tile_pool(name="const", bufs=1) as cpool,
            tc.tile_pool(name="xin", bufs=4) as xpool,
            tc.tile_pool(name="oout", bufs=4) as opool,
        ):
            s_sb = cpool.tile([P, NFT * nslots], f32, tag="stab")
            t_sb = cpool.tile([P, NFT * nslots], f32, tag="ttab")
            scr = cpool.tile([P, R], bf16, tag="scratch")
            nc.sync.dma_start(out=s_sb[:], in_=s_d[:])
            nc.scalar.dma_start(out=t_sb[:], in_=t_d[:])

            xts = {}

            def emit_load(ft):
                ld = nc.sync if ft % 2 == 0 else nc.scalar
                rows = slice(ft * P, (ft + 1) * P)
                xt = xpool.tile([P, R], bf16, tag="x")
                if ft < 2:
                    # split the first load on each ring: compute starts
                    # after half a tile is in
                    ld.dma_start(out=xt[:, 0:h], in_=x_d[rows, 0:h])
                    ld.dma_start(out=xt[:, h:], in_=x_d[rows, h:])
                else:
                    ld.dma_start(out=xt[:], in_=x_d[rows, :])
                xts[ft] = xt

            def emit_compute_store(ft):
                st = nc.scalar if ft % 2 == 0 else nc.sync
                rows = slice(ft * P, (ft + 1) * P)
                xt = xts.pop(ft)
                ot = opool.tile([P, R], bf16, tag="o")
                for j in range(nslots):
                    cs = slice(q[j], q[j + 1])
                    col = ft * nslots + j
                    nc.vector.tensor_scalar(
                        ot[:, cs], xt[:, cs],
                        s_sb[:, col:col + 1], t_sb[:, col:col + 1],
                        mult, add)
                if ft >= NFT - 2:
                    # split the last store on each ring: the drain is
                    # latency-bound once loads have finished
                    st.dma_start(out=o_d[rows, 0:h], in_=ot[:, 0:h])
                    st.dma_start(out=o_d[rows, h:], in_=ot[:, h:])
                else:
                    st.dma_start(out=o_d[rows, :], in_=ot[:])
                # Pacing: the two NeuronCores sharing an HBM stack issue
                # identical demand, but arbitration favors one of them
                # (~400 vs ~330 GB/s observed).  Since the score is the
                # slowest core, cap every core's pipeline cadence near the
                # fair-share rate (~716/2 GB/s) with dummy VectorE ops that
                # keep the x tile live: the pool then recycles at the paced
                # rate, the "winner" stops over-demanding, and its
                # stack-mate gets its half.  Skipped on the first/last
                # tiles so the head fills and the tail drains greedily.
                if 1 <= ft <= NFT - 2:
                    for _ in range(2):
                        nc.vector.tensor_scalar(
                            scr[:], xt[:],
                            s_sb[:, 0:1], t_sb[:, 0:1], mult, add)

            for ft in range(NFT + 2):
                if ft < NFT:
                    emit_load(ft)
                if ft >= 2:
                    emit_compute_store(ft - 2)
    nc.compile()
    return nc


def _host_tables(Wg, Wb, bn_gamma, bn_beta, moving_mean, moving_var):
    inv = (bn_gamma.astype(np.float64)
           / np.sqrt(moving_var.astype(np.float64) + BN_EPS))
    gp1 = 1.0 + Wg.astype(np.float64)  # [C, F]
    S = (inv[None, :] * gp1).astype(np.float32)
    T = ((bn_beta.astype(np.float64) - moving_mean.astype(np.float64) * inv)[None, :]
         * gp1 + Wb.astype(np.float64)).astype(np.float32)
    return S, T


def kernel(x, Wg, Wb, bn_gamma, bn_beta, moving_mean, moving_var, cluster_ids):
    global _PROGRAM, _PROG_KEY
    import ml_dtypes
    from concourse.bass_utils import run_bass_kernel_spmd

    bf16 = ml_dtypes.bfloat16
    x = np.asarray(x, dtype=np.float32)
    ids = np.asarray(cluster_ids, dtype=np.int32)
    S, T = _host_tables(
        np.asarray(Wg, np.float32), np.asarray(Wb, np.float32),
        np.asarray(bn_gamma, np.float32), np.asarray(bn_beta, np.float32),
        np.asarray(moving_mean, np.float32), np.asarray(moving_var, np.float32),
    )

    counts = np.bincount(ids, minlength=C)
    present = np.nonzero(counts)[0]
    ranked = present[np.argsort(-counts[present], kind="stable")]
    npad = (-len(ranked)) % N_CORES
    ranked = np.concatenate([ranked, np.full(npad, -1, dtype=np.int64)])
    nslots = len(ranked) // N_CORES
    slot_cl = ranked.reshape(nslots, N_CORES)   # [slot, core] -> cluster id
    # slot extents: max cluster size in the octile, rounded up to even
    M = []
    for j in range(nslots):
        mx = max((int(counts[cl]) for cl in slot_cl[j] if cl >= 0), default=0)
        M.append(max(2, ((mx + 1) // 2) * 2))
    R = int(sum(M))
    q = [0]
    for m in M:
        q.append(q[-1] + m)

    order = np.argsort(ids, kind="stable")
    starts = np.zeros(C + 1, dtype=np.int64)
    np.cumsum(counts, out=starts[1:])

    in_maps = []
    idx_all, cnt_all = [], []
    for c in range(N_CORES):
        idx = np.empty(R, dtype=np.int64)
        cnts = np.zeros(nslots, dtype=np.int64)
        stab = np.zeros((F, nslots), dtype=np.float32)
        ttab = np.zeros((F, nslots), dtype=np.float32)
        for j in range(nslots):
            a, b = q[j], q[j + 1]
            cl = int(slot_cl[j, c])
            if cl < 0:
                idx[a:b] = order[0]     # scale/bias stay 0; never scattered
                continue
            n = int(counts[cl])
            rows = order[starts[cl]:starts[cl] + n]
            idx[a:a + n] = rows
            idx[a + n:b] = rows[0]      # pad with a row of the same cluster
            cnts[j] = n
            stab[:, j] = S[cl]
            ttab[:, j] = T[cl]
        idx_all.append(idx)
        cnt_all.append(cnts)
        # feature-major bf16 transpose of this core's rows: [F, R]
        xt = x[idx].T.astype(bf16, order="C")
        st = stab.reshape(NFT, P, nslots).transpose(1, 0, 2).reshape(P, NFT * nslots)
        tt = ttab.reshape(NFT, P, nslots).transpose(1, 0, 2).reshape(P, NFT * nslots)
        in_maps.append({
            "xt": xt,
            "stab": np.ascontiguousarray(st),
            "ttab": np.ascontiguousarray(tt),
        })

    key = (R, tuple(M))
    if _PROGRAM is None or _PROG_KEY != key:
        _PROGRAM = _build_program(R, M)
        _PROG_KEY = key

    res = run_bass_kernel_spmd(_PROGRAM, in_maps, list(range(N_CORES)))
    globals()["LAST_RESULT"] = res

    out = np.empty((B, F), dtype=np.float32)
    for c in range(N_CORES):
        z = np.asarray(res.results[c]["ot"])          # [F, R] bf16
        zf = z.T.astype(np.float32, order="C")        # [R, F]
        idx, cnts = idx_all[c], cnt_all[c]
        for j in range(nslots):
            a, n = q[j], int(cnts[j])
            if n:
                out[idx[a:a + n]] = zf[a:a + n]
    return out


if __name__ == "__main__":
    # Smoke test with random data against a local numpy reference.
    rng = np.random.default_rng(0)
    inputs = {
        "x": rng.standard_normal((B, F), dtype=np.float32),
        "Wg": 0.25 * rng.standard_normal((C, F)).astype(np.float32),
        "Wb": 0.25 * rng.standard_normal((C, F)).astype(np.float32),
        "bn_gamma": np.ones(F, np.float32),
        "bn_beta": np.zeros(F, np.float32),
        "moving_mean": 0.1 * rng.standard_normal(F).astype(np.float32),
        "moving_var": rng.uniform(0.5, 1.5, F).astype(np.float32),
        "cluster_ids": rng.integers(0, C, B, dtype=np.int32),
    }
    out = kernel(**inputs)
    inv = inputs["bn_gamma"] / np.sqrt(inputs["moving_var"] + BN_EPS)
    xn = (inputs["x"] - inputs["moving_mean"]) * inv + inputs["bn_beta"]
    g = inputs["Wg"][inputs["cluster_ids"]]
    b = inputs["Wb"][inputs["cluster_ids"]]
    ref = xn * (1.0 + g) + b
    err = np.max(np.abs(out - ref)) / np.max(np.abs(ref))
    print("rel err:", err)
